# revision 38
# baseline (speedup 1.0000x reference)
"""Embedding lookup kernel for Trainium2 (8 NeuronCores, SPMD data-parallel).

Problem: out[b, s, :] = table[ids[b, s], :]
  ids:   [32, 8192] int32 (values in [0, 256))
  table: [256, 256] float32
  out:   [32, 8192, 256] float32

Strategy (data-parallel over tokens per the sharding hint):
  - 262144 tokens split into 8 contiguous shards of 32768 tokens (4 batch
    rows per core); the 256 KiB table is replicated to every core's SBUF.
  - Per core the gather is computed as one_hot(ids) @ table on the PE so
    the table is only read from SBUF — HBM traffic is just ids in
    (128 KiB) + output out (32 MiB), the memory-roofline minimum.
    Per 512-token group: PE broadcasts ids across the 128 partitions
    (K=1 ones-matmul, float32r), DVE builds the transposed one-hot with
    per-partition iota is_equal compares, PE contracts
    one_hot[voc,tok].T @ table[voc,emb] in float32r (full-rate fp32),
    ACT copies PSUM->SBUF, and HWDGE streams 512 KiB contiguous stores.
  - float32r rounds the table to ~13 mantissa bits: rel err ~1.6e-4.
    _build_program_b(hilo=True) adds a residual-correction pass
    (bit-exact vs the fp32 reference) at ~1.5x the PE cost.
  - `_build_program` is an alternative pure-DMA implementation (SWDGE
    dma_gather from the HBM table): exact but ~2.2x slower (reads the
    table rows from HBM, doubling HBM traffic). Kept as fallback.
  - Host-side prep (cheap numpy): ids are transposed so each partition
    owns a contiguous run of output tokens, making every store fully
    contiguous per partition.
"""

import sys

if "/opt/trn_rl_repo" not in sys.path:
    sys.path.insert(0, "/opt/trn_rl_repo")

import numpy as np

BATCH, SEQ, VOCAB, EMBED = 32, 8192, 256, 256
N_CORES = 8
TOKENS = BATCH * SEQ                 # 262144
TOK_PER_CORE = TOKENS // N_CORES     # 32768
P = 128                              # SBUF partitions
TPP = TOK_PER_CORE // P              # tokens per partition = 256
NCHUNK = 32
CHUNK_TOK = TOK_PER_CORE // NCHUNK   # 1024 (SWDGE ring: <= scratch/16 descs/gather)
CHUNK_COLS = CHUNK_TOK // P          # 8 tokens per partition per chunk
NBUF = 3
DMA_SCRATCH = 49152                  # 3072-descriptor SWDGE ring

_CACHE = {}


def _build_program(repeats: int = 1):
    import concourse.bass as bass
    import concourse.mybir as mybir
    from concourse import bacc

    nc = bacc.Bacc("TRN2", target_bir_lowering=False, debug=False,
                   num_devices=N_CORES, dynamic_dma_scratch_size=DMA_SCRATCH)

    idx_d = nc.dram_tensor("idx", [P, TOK_PER_CORE // 16], mybir.dt.int16,
                           kind="ExternalInput")
    table_d = nc.dram_tensor("table", [VOCAB, EMBED], mybir.dt.float32,
                             kind="ExternalInput")
    out_d = nc.dram_tensor("out", [P, TPP * EMBED], mybir.dt.float32,
                           kind="ExternalOutput")

    with nc.Block() as block:
        idx_sb = nc.alloc_sbuf_tensor("idx_sb", [P, TOK_PER_CORE // 16],
                                      mybir.dt.int16)
        gbufs = [
            nc.alloc_sbuf_tensor(f"gbuf{b}", [P, CHUNK_COLS * EMBED],
                                 mybir.dt.float32)
            for b in range(NBUF)
        ]
        isem = nc.alloc_semaphore("isem")
        gsems = [nc.alloc_semaphore(f"gsem{b}") for b in range(NBUF)]
        osems = [nc.alloc_semaphore(f"osem{b}") for b in range(NBUF)]

        total = NCHUNK * repeats

        @block.gpsimd
        def _(gpsimd):
            gpsimd.dma_start(idx_sb.ap(), idx_d.ap()).then_inc(isem, 16)
            gpsimd.wait_ge(isem, 16)
            for k in range(total):
                b, m = k % NBUF, k // NBUF
                kk = k % NCHUNK
                if k >= NBUF:
                    # buffer b is free once chunk k-NBUF's store finished
                    gpsimd.wait_ge(osems[b], 16 * m)
                gb = gbufs[b]
                out_view = gb.ap().rearrange("p (c e) -> p c e", e=EMBED)
                idx_slice = idx_sb.ap()[:, kk * (CHUNK_TOK // 16):(kk + 1) * (CHUNK_TOK // 16)]
                gpsimd.dma_gather(
                    out_view,
                    table_d.ap(),
                    idx_slice,
                    num_idxs=CHUNK_TOK,
                    num_idxs_reg=CHUNK_TOK,
                    elem_size=EMBED,
                ).then_inc(gsems[b], 16)

        @block.sync
        def _(sync):
            for k in range(total):
                b, m = k % NBUF, k // NBUF
                kk = k % NCHUNK
                sync.wait_ge(gsems[b], 16 * (m + 1))
                sync.dma_start(
                    out_d.ap()[:, kk * CHUNK_COLS * EMBED:(kk + 1) * CHUNK_COLS * EMBED],
                    gbufs[b].ap(),
                ).then_inc(osems[b], 16)
            for b in range(NBUF):
                sync.wait_ge(osems[b], 16 * ((total + NBUF - 1 - b) // NBUF))

    nc.compile()
    return nc


def _build_program_b(repeats: int = 1, hilo: bool = False, ps_blocks: int = 2,
                     bc_bufs: int = 3, ps_bufs: int = 5, oh_bufs: int = 6,
                     ob_bufs: int = 4, sg: int = 1, wdt: str = "f32r",
                     gt: int = 512, idsf_split: int = 4):
    """Plan B: one-hot @ table matmul from an SBUF-resident table.

    Per 512-token group: PE broadcasts ids across partitions (K=1 f32r
    matmul into PSUM), DVE builds the transposed one-hot with per-partition
    iota compares (f32r), PE contracts one-hot @ table (f32r, optionally a
    second hi/lo residual pass for ~1e-8 accuracy), ACT copies PSUM->SBUF,
    HWDGE streams results out. Only HBM traffic: ids in (128 KiB) + out
    (32 MiB) per core.
    """
    import concourse.bass as bass
    import concourse.mybir as mybir
    import concourse.tile as tile
    from concourse import bacc

    f32, f32r = mybir.dt.float32, mybir.dt.float32r
    mdt = {"f32r": f32r, "bf16": mybir.dt.bfloat16}[wdt]
    GT = gt                       # tokens per group (512 = 1 PSUM bank)
    NG = TOK_PER_CORE // GT
    BPG = GT // P                 # blocks per group

    nc = bacc.Bacc("TRN2", target_bir_lowering=False, debug=False,
                   num_devices=N_CORES)

    idsf_d = nc.dram_tensor("idsf", [1, TOK_PER_CORE], f32r,
                            kind="ExternalInput")
    table_d = nc.dram_tensor("table", [VOCAB, EMBED], f32,
                             kind="ExternalInput")
    ones_d = nc.dram_tensor("ones", [1, P], f32r, kind="ExternalInput")
    iota_d = nc.dram_tensor("iota", [P, 2], f32, kind="ExternalInput")
    out_d = nc.dram_tensor("out", [P, TPP * EMBED], mybir.dt.float32,
                           kind="ExternalOutput")

    with tile.TileContext(nc) as tc:
        with (
            tc.tile_pool(name="const", bufs=1) as const,
            tc.tile_pool(name="ohp", bufs=oh_bufs) as ohp,
            tc.tile_pool(name="obp", bufs=ob_bufs) as obp,
            tc.tile_pool(name="bcp", bufs=bc_bufs, space="PSUM") as bcp,
            tc.tile_pool(name="psp", bufs=ps_bufs, space="PSUM") as psp,
        ):
            idsf_chunk = TOK_PER_CORE // idsf_split
            idsf_tiles = []
            for q in range(idsf_split):
                t = const.tile([1, idsf_chunk], f32r, tag=f"idsf{q}")
                nc.sync.dma_start(
                    t[:], idsf_d.ap()[:, q * idsf_chunk:(q + 1) * idsf_chunk])
                idsf_tiles.append(t)

            def idsf_slice(lo, hi):
                q = lo // idsf_chunk
                assert hi <= (q + 1) * idsf_chunk
                return idsf_tiles[q][:, lo - q * idsf_chunk:hi - q * idsf_chunk]
            ones = const.tile([1, P], f32r)
            nc.sync.dma_start(ones[:], ones_d.ap())
            iota2 = const.tile([P, 2], f32)
            nc.sync.dma_start(iota2[:], iota_d.ap())
            tb_raw = const.tile([P, 2 * EMBED], f32)
            nc.sync.dma_start(tb_raw[:, 0:EMBED], table_d.ap()[0:P, :])
            nc.sync.dma_start(tb_raw[:, EMBED:2 * EMBED], table_d.ap()[P:VOCAB, :])
            tb_hi = const.tile([P, 2 * EMBED], mdt)
            nc.vector.tensor_copy(tb_hi[:], tb_raw[:])
            if hilo:
                tb_lo = const.tile([P, 2 * EMBED], mdt)
                # residual = raw - hi, rounded to the matmul dtype
                hi_back = const.tile([P, 2 * EMBED], f32)
                nc.vector.tensor_copy(hi_back[:], tb_hi[:])
                nc.vector.tensor_tensor(tb_lo[:], tb_raw[:], hi_back[:],
                                        mybir.AluOpType.subtract)

            def one_pass():
                ob = None
                for g in range(NG):
                    bc = bcp.tile([P, GT], mybir.dt.float32, tag="bc")
                    for k in range(GT // 512):
                        nc.tensor.matmul(
                            bc[:, k * 512:(k + 1) * 512], ones[:],
                            idsf_slice(g * GT + k * 512, g * GT + (k + 1) * 512),
                            start=True, stop=True)
                    oh = ohp.tile([P, 2 * GT], mdt, tag="oh")
                    nc.vector.tensor_scalar(oh[:, 0:GT], bc[:],
                                            iota2[:, 0:1], None,
                                            mybir.AluOpType.is_equal)
                    nc.vector.tensor_scalar(oh[:, GT:2 * GT], bc[:],
                                            iota2[:, 1:2], None,
                                            mybir.AluOpType.is_equal)
                    nonlocal_ofs = (g % sg) * BPG * EMBED
                    if g % sg == 0:
                        ob = obp.tile([P, sg * BPG * EMBED], mybir.dt.float32,
                                      tag="ob")
                    for hh in range(BPG // ps_blocks):
                        ps = psp.tile([P, ps_blocks * EMBED],
                                      mybir.dt.float32, tag="ps")
                        for jj in range(ps_blocks):
                            j = hh * ps_blocks + jj
                            s = j * P
                            mms = [(0, tb_hi), (1, tb_hi)]
                            if hilo:
                                mms += [(0, tb_lo), (1, tb_lo)]
                            for mi, (v, tbl) in enumerate(mms):
                                nc.tensor.matmul(
                                    ps[:, jj * EMBED:(jj + 1) * EMBED],
                                    oh[:, v * GT + s: v * GT + s + P],
                                    tbl[:, v * EMBED:(v + 1) * EMBED],
                                    start=(mi == 0), stop=(mi == len(mms) - 1),
                                )
                        nc.scalar.copy(
                            ob[:, nonlocal_ofs + hh * ps_blocks * EMBED:
                               nonlocal_ofs + (hh + 1) * ps_blocks * EMBED],
                            ps[:])
                    if g % sg == sg - 1:
                        g0 = g - (sg - 1)
                        nc.sync.dma_start(
                            out_d.ap()[:, g0 * BPG * EMBED:(g + 1) * BPG * EMBED],
                            ob[:])

            if repeats == 1:
                one_pass()
            else:
                with tc.For_i(0, repeats, 1):
                    one_pass()

    nc.compile()
    return nc


def _prep_idsf(shard_ids: np.ndarray) -> np.ndarray:
    """[32768] ids -> [1, 32768] f32 in interleaved feed order."""
    fed = shard_ids.reshape(P, TPP).T.reshape(1, -1)
    return np.ascontiguousarray(fed.astype(np.float32))


def _plan_b_consts():
    iota = np.stack([np.arange(P, dtype=np.float32),
                     np.arange(P, 2 * P, dtype=np.float32)], axis=1)
    return {
        "ones": np.ones((1, P), np.float32),
        "iota": np.ascontiguousarray(iota),
    }


def _prep_idx(shard_ids: np.ndarray) -> np.ndarray:
    """shard_ids: [32768] int -> [128, 2048] int16 in gather feed order.

    Feed order: fed[i] = shard[(i % 128) * TPP + i // 128] so the gather
    (which writes token i to partition i%128, column i//128) leaves each
    partition holding a contiguous run of TPP output tokens.
    Wrapped %16 across partitions, replicated into all 8 16-partition groups.
    """
    fed = shard_ids.reshape(P, TPP).T.reshape(-1)          # [32768]
    t16 = fed.reshape(TOK_PER_CORE // 16, 16).T            # [16, 2048]
    return np.ascontiguousarray(np.tile(t16, (8, 1)).astype(np.int16))


def kernel(inputs: np.ndarray, kernel: np.ndarray) -> np.ndarray:
    from concourse.bass_utils import run_bass_kernel_spmd

    ids = np.asarray(inputs, dtype=np.int32).reshape(-1)
    table = np.ascontiguousarray(np.asarray(kernel, dtype=np.float32))

    if "nc" not in _CACHE:
        _CACHE["nc"] = _build_program_b(1)
    nc = _CACHE["nc"]

    consts = _plan_b_consts()
    in_maps = []
    for c in range(N_CORES):
        shard = ids[c * TOK_PER_CORE:(c + 1) * TOK_PER_CORE]
        in_maps.append({"idsf": _prep_idsf(shard), "table": table, **consts})

    res = run_bass_kernel_spmd(nc, in_maps, core_ids=list(range(N_CORES)))
    _CACHE["last_results"] = res

    parts = []
    for c in range(N_CORES):
        o = res.results[c]["out"]                  # [128, 65536] f32
        parts.append(o.reshape(TOK_PER_CORE, EMBED))
    return np.concatenate(parts, axis=0).reshape(BATCH, SEQ, EMBED)


# revision 40
# speedup vs baseline: 1.1883x; 1.1883x over previous
"""Embedding lookup kernel for Trainium2 (8 NeuronCores, SPMD data-parallel).

Problem: out[b, s, :] = table[ids[b, s], :]
  ids:   [32, 8192] int32 (values in [0, 256))
  table: [256, 256] float32
  out:   [32, 8192, 256] float32

Strategy (data-parallel over tokens per the sharding hint):
  - 262144 tokens split into 8 contiguous shards of 32768 tokens (4 batch
    rows per core); the 256 KiB table is replicated to every core's SBUF.
  - Per core the gather is computed as one_hot(ids) @ table on the PE so
    the table is only read from SBUF — HBM traffic is just ids in
    (128 KiB) + output out (32 MiB), the memory-roofline minimum.
    Per 512-token group: PE broadcasts ids across the 128 partitions
    (K=1 ones-matmul, float32r), DVE builds the transposed one-hot with
    per-partition iota is_equal compares, PE contracts
    one_hot[voc,tok].T @ table[voc,emb] in float32r (full-rate fp32),
    ACT copies PSUM->SBUF, and HWDGE streams 512 KiB contiguous stores.
  - float32r rounds the table to ~13 mantissa bits: rel err ~1.6e-4.
    _build_program_b(hilo=True) adds a residual-correction pass
    (bit-exact vs the fp32 reference) at ~1.5x the PE cost.
  - `_build_program` is an alternative pure-DMA implementation (SWDGE
    dma_gather from the HBM table): exact but ~2.2x slower (reads the
    table rows from HBM, doubling HBM traffic). Kept as fallback.
  - Host-side prep (cheap numpy): ids are transposed so each partition
    owns a contiguous run of output tokens, making every store fully
    contiguous per partition.
"""

import sys

if "/opt/trn_rl_repo" not in sys.path:
    sys.path.insert(0, "/opt/trn_rl_repo")

import numpy as np

BATCH, SEQ, VOCAB, EMBED = 32, 8192, 256, 256
N_CORES = 8
TOKENS = BATCH * SEQ                 # 262144
TOK_PER_CORE = TOKENS // N_CORES     # 32768
P = 128                              # SBUF partitions
TPP = TOK_PER_CORE // P              # tokens per partition = 256
NCHUNK = 32
CHUNK_TOK = TOK_PER_CORE // NCHUNK   # 1024 (SWDGE ring: <= scratch/16 descs/gather)
CHUNK_COLS = CHUNK_TOK // P          # 8 tokens per partition per chunk
NBUF = 3
DMA_SCRATCH = 49152                  # 3072-descriptor SWDGE ring

_CACHE = {}


def _build_program(repeats: int = 1):
    import concourse.bass as bass
    import concourse.mybir as mybir
    from concourse import bacc

    nc = bacc.Bacc("TRN2", target_bir_lowering=False, debug=False,
                   num_devices=N_CORES, dynamic_dma_scratch_size=DMA_SCRATCH)

    idx_d = nc.dram_tensor("idx", [P, TOK_PER_CORE // 16], mybir.dt.int16,
                           kind="ExternalInput")
    table_d = nc.dram_tensor("table", [VOCAB, EMBED], mybir.dt.float32,
                             kind="ExternalInput")
    out_d = nc.dram_tensor("out", [P, TPP * EMBED], mybir.dt.float32,
                           kind="ExternalOutput")

    with nc.Block() as block:
        idx_sb = nc.alloc_sbuf_tensor("idx_sb", [P, TOK_PER_CORE // 16],
                                      mybir.dt.int16)
        gbufs = [
            nc.alloc_sbuf_tensor(f"gbuf{b}", [P, CHUNK_COLS * EMBED],
                                 mybir.dt.float32)
            for b in range(NBUF)
        ]
        isem = nc.alloc_semaphore("isem")
        gsems = [nc.alloc_semaphore(f"gsem{b}") for b in range(NBUF)]
        osems = [nc.alloc_semaphore(f"osem{b}") for b in range(NBUF)]

        total = NCHUNK * repeats

        @block.gpsimd
        def _(gpsimd):
            gpsimd.dma_start(idx_sb.ap(), idx_d.ap()).then_inc(isem, 16)
            gpsimd.wait_ge(isem, 16)
            for k in range(total):
                b, m = k % NBUF, k // NBUF
                kk = k % NCHUNK
                if k >= NBUF:
                    # buffer b is free once chunk k-NBUF's store finished
                    gpsimd.wait_ge(osems[b], 16 * m)
                gb = gbufs[b]
                out_view = gb.ap().rearrange("p (c e) -> p c e", e=EMBED)
                idx_slice = idx_sb.ap()[:, kk * (CHUNK_TOK // 16):(kk + 1) * (CHUNK_TOK // 16)]
                gpsimd.dma_gather(
                    out_view,
                    table_d.ap(),
                    idx_slice,
                    num_idxs=CHUNK_TOK,
                    num_idxs_reg=CHUNK_TOK,
                    elem_size=EMBED,
                ).then_inc(gsems[b], 16)

        @block.sync
        def _(sync):
            for k in range(total):
                b, m = k % NBUF, k // NBUF
                kk = k % NCHUNK
                sync.wait_ge(gsems[b], 16 * (m + 1))
                sync.dma_start(
                    out_d.ap()[:, kk * CHUNK_COLS * EMBED:(kk + 1) * CHUNK_COLS * EMBED],
                    gbufs[b].ap(),
                ).then_inc(osems[b], 16)
            for b in range(NBUF):
                sync.wait_ge(osems[b], 16 * ((total + NBUF - 1 - b) // NBUF))

    nc.compile()
    return nc


def _build_program_b(repeats: int = 1, hilo: bool = False, ps_blocks: int = 2,
                     bc_bufs: int = 3, ps_bufs: int = 5, oh_bufs: int = 6,
                     ob_bufs: int = 4, sg: int = 1, wdt: str = "f32r",
                     gt: int = 512, idsf_split: int = 4,
                     dual_ring: bool = False):
    """Plan B: one-hot @ table matmul from an SBUF-resident table.

    Per 512-token group: PE broadcasts ids across partitions (K=1 f32r
    matmul into PSUM), DVE builds the transposed one-hot with per-partition
    iota compares (f32r), PE contracts one-hot @ table (f32r, optionally a
    second hi/lo residual pass for ~1e-8 accuracy), ACT copies PSUM->SBUF,
    HWDGE streams results out. Only HBM traffic: ids in (128 KiB) + out
    (32 MiB) per core.
    """
    import concourse.bass as bass
    import concourse.mybir as mybir
    import concourse.tile as tile
    from concourse import bacc

    f32, f32r = mybir.dt.float32, mybir.dt.float32r
    mdt = {"f32r": f32r, "bf16": mybir.dt.bfloat16}[wdt]
    GT = gt                       # tokens per group (512 = 1 PSUM bank)
    NG = TOK_PER_CORE // GT
    BPG = GT // P                 # blocks per group

    nc = bacc.Bacc("TRN2", target_bir_lowering=False, debug=False,
                   num_devices=N_CORES)

    idsf_d = nc.dram_tensor("idsf", [1, TOK_PER_CORE], f32r,
                            kind="ExternalInput")
    table_d = nc.dram_tensor("table", [VOCAB, EMBED], f32,
                             kind="ExternalInput")
    ones_d = nc.dram_tensor("ones", [1, P], f32r, kind="ExternalInput")
    iota_d = nc.dram_tensor("iota", [P, 2], f32, kind="ExternalInput")
    out_d = nc.dram_tensor("out", [P, TPP * EMBED], mybir.dt.float32,
                           kind="ExternalOutput")

    with tile.TileContext(nc) as tc:
        with (
            tc.tile_pool(name="const", bufs=1) as const,
            tc.tile_pool(name="ohp", bufs=oh_bufs) as ohp,
            tc.tile_pool(name="obp", bufs=ob_bufs) as obp,
            tc.tile_pool(name="bcp", bufs=bc_bufs, space="PSUM") as bcp,
            tc.tile_pool(name="psp", bufs=ps_bufs, space="PSUM") as psp,
        ):
            idsf_chunk = TOK_PER_CORE // idsf_split
            idsf_tiles = []
            for q in range(idsf_split):
                t = const.tile([1, idsf_chunk], f32r, tag=f"idsf{q}")
                nc.sync.dma_start(
                    t[:], idsf_d.ap()[:, q * idsf_chunk:(q + 1) * idsf_chunk])
                idsf_tiles.append(t)

            def idsf_slice(lo, hi):
                q = lo // idsf_chunk
                assert hi <= (q + 1) * idsf_chunk
                return idsf_tiles[q][:, lo - q * idsf_chunk:hi - q * idsf_chunk]
            ones = const.tile([1, P], f32r)
            nc.sync.dma_start(ones[:], ones_d.ap())
            iota2 = const.tile([P, 2], f32)
            nc.sync.dma_start(iota2[:], iota_d.ap())
            tb_raw = const.tile([P, 2 * EMBED], f32)
            nc.sync.dma_start(tb_raw[:, 0:EMBED], table_d.ap()[0:P, :])
            nc.sync.dma_start(tb_raw[:, EMBED:2 * EMBED], table_d.ap()[P:VOCAB, :])
            tb_hi = const.tile([P, 2 * EMBED], mdt)
            nc.vector.tensor_copy(tb_hi[:], tb_raw[:])
            if hilo:
                tb_lo = const.tile([P, 2 * EMBED], mdt)
                # residual = raw - hi, rounded to the matmul dtype
                hi_back = const.tile([P, 2 * EMBED], f32)
                nc.vector.tensor_copy(hi_back[:], tb_hi[:])
                nc.vector.tensor_tensor(tb_lo[:], tb_raw[:], hi_back[:],
                                        mybir.AluOpType.subtract)

            def one_pass():
                ob = None
                for g in range(NG):
                    bc = bcp.tile([P, GT], mybir.dt.float32, tag="bc")
                    for k in range(GT // 512):
                        nc.tensor.matmul(
                            bc[:, k * 512:(k + 1) * 512], ones[:],
                            idsf_slice(g * GT + k * 512, g * GT + (k + 1) * 512),
                            start=True, stop=True)
                    oh = ohp.tile([P, 2 * GT], mdt, tag="oh")
                    nc.vector.tensor_scalar(oh[:, 0:GT], bc[:],
                                            iota2[:, 0:1], None,
                                            mybir.AluOpType.is_equal)
                    nc.vector.tensor_scalar(oh[:, GT:2 * GT], bc[:],
                                            iota2[:, 1:2], None,
                                            mybir.AluOpType.is_equal)
                    nonlocal_ofs = (g % sg) * BPG * EMBED
                    if g % sg == 0:
                        ob = obp.tile([P, sg * BPG * EMBED], mybir.dt.float32,
                                      tag="ob")
                    for hh in range(BPG // ps_blocks):
                        ps = psp.tile([P, ps_blocks * EMBED],
                                      mybir.dt.float32, tag="ps")
                        for jj in range(ps_blocks):
                            j = hh * ps_blocks + jj
                            s = j * P
                            mms = [(0, tb_hi), (1, tb_hi)]
                            if hilo:
                                mms += [(0, tb_lo), (1, tb_lo)]
                            for mi, (v, tbl) in enumerate(mms):
                                nc.tensor.matmul(
                                    ps[:, jj * EMBED:(jj + 1) * EMBED],
                                    oh[:, v * GT + s: v * GT + s + P],
                                    tbl[:, v * EMBED:(v + 1) * EMBED],
                                    start=(mi == 0), stop=(mi == len(mms) - 1),
                                )
                        nc.scalar.copy(
                            ob[:, nonlocal_ofs + hh * ps_blocks * EMBED:
                               nonlocal_ofs + (hh + 1) * ps_blocks * EMBED],
                            ps[:])
                    if g % sg == sg - 1:
                        g0 = g - (sg - 1)
                        # alternate the two physical HWDGE rings (SP / ACT)
                        eng = nc.scalar if (dual_ring and g % 2) else nc.sync
                        eng.dma_start(
                            out_d.ap()[:, g0 * BPG * EMBED:(g + 1) * BPG * EMBED],
                            ob[:])

            if repeats == 1:
                one_pass()
            else:
                with tc.For_i(0, repeats, 1):
                    one_pass()

    nc.compile()
    return nc


def _prep_idsf(shard_ids: np.ndarray) -> np.ndarray:
    """[32768] ids -> [1, 32768] f32 in interleaved feed order."""
    fed = shard_ids.reshape(P, TPP).T.reshape(1, -1)
    return np.ascontiguousarray(fed.astype(np.float32))


def _plan_b_consts():
    iota = np.stack([np.arange(P, dtype=np.float32),
                     np.arange(P, 2 * P, dtype=np.float32)], axis=1)
    return {
        "ones": np.ones((1, P), np.float32),
        "iota": np.ascontiguousarray(iota),
    }


def _prep_idx(shard_ids: np.ndarray) -> np.ndarray:
    """shard_ids: [32768] int -> [128, 2048] int16 in gather feed order.

    Feed order: fed[i] = shard[(i % 128) * TPP + i // 128] so the gather
    (which writes token i to partition i%128, column i//128) leaves each
    partition holding a contiguous run of TPP output tokens.
    Wrapped %16 across partitions, replicated into all 8 16-partition groups.
    """
    fed = shard_ids.reshape(P, TPP).T.reshape(-1)          # [32768]
    t16 = fed.reshape(TOK_PER_CORE // 16, 16).T            # [16, 2048]
    return np.ascontiguousarray(np.tile(t16, (8, 1)).astype(np.int16))


def kernel(inputs: np.ndarray, kernel: np.ndarray) -> np.ndarray:
    from concourse.bass_utils import run_bass_kernel_spmd

    ids = np.asarray(inputs, dtype=np.int32).reshape(-1)
    table = np.ascontiguousarray(np.asarray(kernel, dtype=np.float32))

    if "nc" not in _CACHE:
        _CACHE["nc"] = _build_program_b(1)
    nc = _CACHE["nc"]

    consts = _plan_b_consts()
    in_maps = []
    for c in range(N_CORES):
        shard = ids[c * TOK_PER_CORE:(c + 1) * TOK_PER_CORE]
        in_maps.append({"idsf": _prep_idsf(shard), "table": table, **consts})

    res = run_bass_kernel_spmd(nc, in_maps, core_ids=list(range(N_CORES)))
    _CACHE["last_results"] = res

    parts = []
    for c in range(N_CORES):
        o = res.results[c]["out"]                  # [128, 65536] f32
        parts.append(o.reshape(TOK_PER_CORE, EMBED))
    return np.concatenate(parts, axis=0).reshape(BATCH, SEQ, EMBED)


# revision 46
# speedup vs baseline: 1.3704x; 1.1533x over previous
"""Embedding lookup kernel for Trainium2 (8 NeuronCores, SPMD data-parallel).

Problem: out[b, s, :] = table[ids[b, s], :]
  ids:   [32, 8192] int32 (values in [0, 256))
  table: [256, 256] float32
  out:   [32, 8192, 256] float32

Strategy (data-parallel over tokens per the sharding hint):
  - 262144 tokens split into 8 contiguous shards of 32768 tokens (4 batch
    rows per core); the 256 KiB table is replicated to every core's SBUF.
  - Per core the gather is computed as one_hot(ids) @ table on the PE so
    the table is only read from SBUF — HBM traffic is just ids in
    (128 KiB) + output out (32 MiB), the memory-roofline minimum.
    Per 512-token group: PE broadcasts ids across the 128 partitions
    (K=1 ones-matmul, float32r), DVE builds the transposed one-hot with
    per-partition iota is_equal compares, PE contracts
    one_hot[voc,tok].T @ table[voc,emb] in float32r (full-rate fp32),
    ACT copies PSUM->SBUF, and HWDGE streams 512 KiB contiguous stores.
  - float32r rounds the table to ~13 mantissa bits: rel err ~1.6e-4.
    _build_program_b(hilo=True) adds a residual-correction pass
    (bit-exact vs the fp32 reference) at ~1.5x the PE cost.
  - `_build_program` is an alternative pure-DMA implementation (SWDGE
    dma_gather from the HBM table): exact but ~2.2x slower (reads the
    table rows from HBM, doubling HBM traffic). Kept as fallback.
  - Host-side prep (cheap numpy): ids are transposed so each partition
    owns a contiguous run of output tokens, making every store fully
    contiguous per partition.
"""

import sys

if "/opt/trn_rl_repo" not in sys.path:
    sys.path.insert(0, "/opt/trn_rl_repo")

import numpy as np

BATCH, SEQ, VOCAB, EMBED = 32, 8192, 256, 256
N_CORES = 8
TOKENS = BATCH * SEQ                 # 262144
TOK_PER_CORE = TOKENS // N_CORES     # 32768
P = 128                              # SBUF partitions
TPP = TOK_PER_CORE // P              # tokens per partition = 256
NCHUNK = 32
CHUNK_TOK = TOK_PER_CORE // NCHUNK   # 1024 (SWDGE ring: <= scratch/16 descs/gather)
CHUNK_COLS = CHUNK_TOK // P          # 8 tokens per partition per chunk
NBUF = 3
DMA_SCRATCH = 49152                  # 3072-descriptor SWDGE ring

_CACHE = {}


def _build_program(repeats: int = 1):
    import concourse.bass as bass
    import concourse.mybir as mybir
    from concourse import bacc

    nc = bacc.Bacc("TRN2", target_bir_lowering=False, debug=False,
                   num_devices=N_CORES, dynamic_dma_scratch_size=DMA_SCRATCH)

    idx_d = nc.dram_tensor("idx", [P, TOK_PER_CORE // 16], mybir.dt.int16,
                           kind="ExternalInput")
    table_d = nc.dram_tensor("table", [VOCAB, EMBED], mybir.dt.float32,
                             kind="ExternalInput")
    out_d = nc.dram_tensor("out", [P, TPP * EMBED], mybir.dt.float32,
                           kind="ExternalOutput")

    with nc.Block() as block:
        idx_sb = nc.alloc_sbuf_tensor("idx_sb", [P, TOK_PER_CORE // 16],
                                      mybir.dt.int16)
        gbufs = [
            nc.alloc_sbuf_tensor(f"gbuf{b}", [P, CHUNK_COLS * EMBED],
                                 mybir.dt.float32)
            for b in range(NBUF)
        ]
        isem = nc.alloc_semaphore("isem")
        gsems = [nc.alloc_semaphore(f"gsem{b}") for b in range(NBUF)]
        osems = [nc.alloc_semaphore(f"osem{b}") for b in range(NBUF)]

        total = NCHUNK * repeats

        @block.gpsimd
        def _(gpsimd):
            gpsimd.dma_start(idx_sb.ap(), idx_d.ap()).then_inc(isem, 16)
            gpsimd.wait_ge(isem, 16)
            for k in range(total):
                b, m = k % NBUF, k // NBUF
                kk = k % NCHUNK
                if k >= NBUF:
                    # buffer b is free once chunk k-NBUF's store finished
                    gpsimd.wait_ge(osems[b], 16 * m)
                gb = gbufs[b]
                out_view = gb.ap().rearrange("p (c e) -> p c e", e=EMBED)
                idx_slice = idx_sb.ap()[:, kk * (CHUNK_TOK // 16):(kk + 1) * (CHUNK_TOK // 16)]
                gpsimd.dma_gather(
                    out_view,
                    table_d.ap(),
                    idx_slice,
                    num_idxs=CHUNK_TOK,
                    num_idxs_reg=CHUNK_TOK,
                    elem_size=EMBED,
                ).then_inc(gsems[b], 16)

        @block.sync
        def _(sync):
            for k in range(total):
                b, m = k % NBUF, k // NBUF
                kk = k % NCHUNK
                sync.wait_ge(gsems[b], 16 * (m + 1))
                sync.dma_start(
                    out_d.ap()[:, kk * CHUNK_COLS * EMBED:(kk + 1) * CHUNK_COLS * EMBED],
                    gbufs[b].ap(),
                ).then_inc(osems[b], 16)
            for b in range(NBUF):
                sync.wait_ge(osems[b], 16 * ((total + NBUF - 1 - b) // NBUF))

    nc.compile()
    return nc


def _build_program_b(repeats: int = 1, hilo: bool = False, ps_blocks: int = 2,
                     bc_bufs: int = 3, ps_bufs: int = 8, oh_bufs: int = 6,
                     ob_bufs: int = 4, sg: int = 1, wdt: str = "f32r",
                     gt: int = 512, idsf_split: int = 4,
                     dual_ring: bool = False, bcast_eng: str = "pool"):
    """Plan B: one-hot @ table matmul from an SBUF-resident table.

    Per 512-token group: PE broadcasts ids across partitions (K=1 f32r
    matmul into PSUM), DVE builds the transposed one-hot with per-partition
    iota compares (f32r), PE contracts one-hot @ table (f32r, optionally a
    second hi/lo residual pass for ~1e-8 accuracy), ACT copies PSUM->SBUF,
    HWDGE streams results out. Only HBM traffic: ids in (128 KiB) + out
    (32 MiB) per core.
    """
    import concourse.bass as bass
    import concourse.mybir as mybir
    import concourse.tile as tile
    from concourse import bacc

    f32, f32r = mybir.dt.float32, mybir.dt.float32r
    mdt = {"f32r": f32r, "bf16": mybir.dt.bfloat16}[wdt]
    GT = gt                       # tokens per group (512 = 1 PSUM bank)
    NG = TOK_PER_CORE // GT
    BPG = GT // P                 # blocks per group

    nc = bacc.Bacc("TRN2", target_bir_lowering=False, debug=False,
                   num_devices=N_CORES)

    idsf_dt = f32r if bcast_eng == "pe" else f32
    idsf_d = nc.dram_tensor("idsf", [1, TOK_PER_CORE], idsf_dt,
                            kind="ExternalInput")
    table_d = nc.dram_tensor("table", [VOCAB, EMBED], f32,
                             kind="ExternalInput")
    ones_d = nc.dram_tensor("ones", [1, P], f32r, kind="ExternalInput")
    iota_d = nc.dram_tensor("iota", [P, 2], f32, kind="ExternalInput")
    out_d = nc.dram_tensor("out", [P, TPP * EMBED], mybir.dt.float32,
                           kind="ExternalOutput")

    with tile.TileContext(nc) as tc:
        with (
            tc.tile_pool(name="const", bufs=1) as const,
            tc.tile_pool(name="ohp", bufs=oh_bufs) as ohp,
            tc.tile_pool(name="obp", bufs=ob_bufs) as obp,
            tc.tile_pool(name="bcp", bufs=bc_bufs,
                         space="PSUM" if bcast_eng == "pe" else "SBUF") as bcp,
            tc.tile_pool(name="psp", bufs=ps_bufs, space="PSUM") as psp,
        ):
            idsf_chunk = TOK_PER_CORE // idsf_split
            idsf_tiles = []
            for q in range(idsf_split):
                t = const.tile([1, idsf_chunk], idsf_dt, tag=f"idsf{q}")
                nc.sync.dma_start(
                    t[:], idsf_d.ap()[:, q * idsf_chunk:(q + 1) * idsf_chunk])
                idsf_tiles.append(t)

            def idsf_slice(lo, hi):
                q = lo // idsf_chunk
                assert hi <= (q + 1) * idsf_chunk
                return idsf_tiles[q][:, lo - q * idsf_chunk:hi - q * idsf_chunk]
            ones = const.tile([1, P], f32r)
            nc.sync.dma_start(ones[:], ones_d.ap())
            iota2 = const.tile([P, 2], f32)
            nc.sync.dma_start(iota2[:], iota_d.ap())
            tb_raw = const.tile([P, 2 * EMBED], f32)
            nc.sync.dma_start(tb_raw[:, 0:EMBED], table_d.ap()[0:P, :])
            nc.sync.dma_start(tb_raw[:, EMBED:2 * EMBED], table_d.ap()[P:VOCAB, :])
            tb_hi = const.tile([P, 2 * EMBED], mdt)
            nc.vector.tensor_copy(tb_hi[:], tb_raw[:])
            if hilo:
                tb_lo = const.tile([P, 2 * EMBED], mdt)
                # residual = raw - hi, rounded to the matmul dtype
                hi_back = const.tile([P, 2 * EMBED], f32)
                nc.vector.tensor_copy(hi_back[:], tb_hi[:])
                nc.vector.tensor_tensor(tb_lo[:], tb_raw[:], hi_back[:],
                                        mybir.AluOpType.subtract)

            def one_pass():
                ob = None
                for g in range(NG):
                    bc = bcp.tile([P, GT], mybir.dt.float32, tag="bc")
                    if bcast_eng == "pe":
                        for k in range(GT // 512):
                            nc.tensor.matmul(
                                bc[:, k * 512:(k + 1) * 512], ones[:],
                                idsf_slice(g * GT + k * 512,
                                           g * GT + (k + 1) * 512),
                                start=True, stop=True)
                    else:
                        nc.gpsimd.partition_broadcast(
                            bc[:], idsf_slice(g * GT, (g + 1) * GT).bitcast(
                                mybir.dt.float32))
                    oh = ohp.tile([P, 2 * GT], mdt, tag="oh")
                    nc.vector.tensor_scalar(oh[:, 0:GT], bc[:],
                                            iota2[:, 0:1], None,
                                            mybir.AluOpType.is_equal)
                    nc.vector.tensor_scalar(oh[:, GT:2 * GT], bc[:],
                                            iota2[:, 1:2], None,
                                            mybir.AluOpType.is_equal)
                    nonlocal_ofs = (g % sg) * BPG * EMBED
                    if g % sg == 0:
                        ob = obp.tile([P, sg * BPG * EMBED], mybir.dt.float32,
                                      tag="ob")
                    for hh in range(BPG // ps_blocks):
                        ps = psp.tile([P, ps_blocks * EMBED],
                                      mybir.dt.float32, tag="ps")
                        for jj in range(ps_blocks):
                            j = hh * ps_blocks + jj
                            s = j * P
                            mms = [(0, tb_hi), (1, tb_hi)]
                            if hilo:
                                mms += [(0, tb_lo), (1, tb_lo)]
                            for mi, (v, tbl) in enumerate(mms):
                                nc.tensor.matmul(
                                    ps[:, jj * EMBED:(jj + 1) * EMBED],
                                    oh[:, v * GT + s: v * GT + s + P],
                                    tbl[:, v * EMBED:(v + 1) * EMBED],
                                    start=(mi == 0), stop=(mi == len(mms) - 1),
                                )
                        nc.scalar.copy(
                            ob[:, nonlocal_ofs + hh * ps_blocks * EMBED:
                               nonlocal_ofs + (hh + 1) * ps_blocks * EMBED],
                            ps[:])
                    if g % sg == sg - 1:
                        g0 = g - (sg - 1)
                        # alternate the two physical HWDGE rings (SP / ACT)
                        eng = nc.scalar if (dual_ring and g % 2) else nc.sync
                        eng.dma_start(
                            out_d.ap()[:, g0 * BPG * EMBED:(g + 1) * BPG * EMBED],
                            ob[:])

            if repeats == 1:
                one_pass()
            else:
                with tc.For_i(0, repeats, 1):
                    one_pass()

    nc.compile()
    return nc


def _prep_idsf(shard_ids: np.ndarray) -> np.ndarray:
    """[32768] ids -> [1, 32768] f32 in interleaved feed order."""
    fed = shard_ids.reshape(P, TPP).T.reshape(1, -1)
    return np.ascontiguousarray(fed.astype(np.float32))


def _plan_b_consts():
    iota = np.stack([np.arange(P, dtype=np.float32),
                     np.arange(P, 2 * P, dtype=np.float32)], axis=1)
    return {
        "ones": np.ones((1, P), np.float32),
        "iota": np.ascontiguousarray(iota),
    }


def _prep_idx(shard_ids: np.ndarray) -> np.ndarray:
    """shard_ids: [32768] int -> [128, 2048] int16 in gather feed order.

    Feed order: fed[i] = shard[(i % 128) * TPP + i // 128] so the gather
    (which writes token i to partition i%128, column i//128) leaves each
    partition holding a contiguous run of TPP output tokens.
    Wrapped %16 across partitions, replicated into all 8 16-partition groups.
    """
    fed = shard_ids.reshape(P, TPP).T.reshape(-1)          # [32768]
    t16 = fed.reshape(TOK_PER_CORE // 16, 16).T            # [16, 2048]
    return np.ascontiguousarray(np.tile(t16, (8, 1)).astype(np.int16))


def kernel(inputs: np.ndarray, kernel: np.ndarray) -> np.ndarray:
    from concourse.bass_utils import run_bass_kernel_spmd

    ids = np.asarray(inputs, dtype=np.int32).reshape(-1)
    table = np.ascontiguousarray(np.asarray(kernel, dtype=np.float32))

    if "nc" not in _CACHE:
        _CACHE["nc"] = _build_program_b(1)
    nc = _CACHE["nc"]

    consts = _plan_b_consts()
    in_maps = []
    for c in range(N_CORES):
        shard = ids[c * TOK_PER_CORE:(c + 1) * TOK_PER_CORE]
        in_maps.append({"idsf": _prep_idsf(shard), "table": table, **consts})

    res = run_bass_kernel_spmd(nc, in_maps, core_ids=list(range(N_CORES)))
    _CACHE["last_results"] = res

    parts = []
    for c in range(N_CORES):
        o = res.results[c]["out"]                  # [128, 65536] f32
        parts.append(o.reshape(TOK_PER_CORE, EMBED))
    return np.concatenate(parts, axis=0).reshape(BATCH, SEQ, EMBED)
